# revision 31
# baseline (speedup 1.0000x reference)
"""Multi-head attention (B=2, S=2048, E=1024, H=16, causal) on 8 TRN2 cores.

Sharding: core c -> batch b = c//4, head group g = c%4 (4 heads each).
Each core computes QKV projection for its heads, causal flash-style
attention (no-max softmax, denominator via ones-column appended to V),
and a partial output projection against a 256-row slice of W_proj.
Host sums the 4 partial projections per batch (the "all-reduce") and
stacks the 2 batches.

All matmul operands are bfloat16 (fp32 accumulation in PSUM): same PE
row rate as fp32r but ~2x faster LDWEIGHTS, half the DMA bytes, half
SBUF pressure, and 2x DVE throughput on copies.

Scheduling: attention is ACT(exp)-latency-bound per score tile, and the
PE queue is in-order — so independent QKV/projection matmul work is
chopped into ~1-2us micro-pieces and interleaved after (almost) every
attention k-tile, keeping the PE dense and HAM-warm.  The final
attention chunk splits its attV accumulation into two q-halves so the
softmax-normalize chain (PSUM row copy -> gpsimd partition broadcast ->
reciprocal -> scale) of each half hides under remaining matmul work.
"""
import sys

sys.path.insert(0, "/opt/trn_rl_repo")

import numpy as np
import ml_dtypes

import concourse.bacc as bacc
import concourse.mybir as mybir
from concourse import tile
from concourse.bass_utils import run_bass_kernel_spmd

B, S, E, H, D = 2, 2048, 1024, 16, 64
SCALE = D ** -0.5
N_CORES = 8
HL = 4            # heads per core
GC = 256          # channel columns per core (HL * D)
F32 = mybir.dt.float32
BF16 = mybir.dt.bfloat16
NP_BF16 = ml_dtypes.bfloat16

_CACHED_NC = None

N_WARM = 12       # warmup matmuls to ramp PE p-state during the DMA head


def _build():
    nc = bacc.Bacc("TRN2", target_bir_lowering=False, debug=False,
                   num_devices=N_CORES)

    xT = nc.dram_tensor("xT", [E, S], BF16, kind="ExternalInput")
    w = nc.dram_tensor("w", [E, 3 * GC], BF16, kind="ExternalInput")
    wp = nc.dram_tensor("wp", [GC, E], BF16, kind="ExternalInput")
    mask = nc.dram_tensor("mask", [128, 128], BF16, kind="ExternalInput")
    y = nc.dram_tensor("y", [S, E], BF16, kind="ExternalOutput")

    ET = E // 128     # 8 e-tiles
    NS = S // 512     # 4 s-chunks of 512
    NT = S // 128     # 16 s-tiles of 128

    with tile.TileContext(nc) as tc:
        with (
            tc.tile_pool(name="const", bufs=1) as cst,
            tc.tile_pool(name="acts", bufs=1) as acts,
            tc.tile_pool(name="expp", bufs=8) as expp,
            tc.tile_pool(name="small", bufs=2) as small,
            tc.tile_pool(name="ysb", bufs=3) as ysbp,
            tc.tile_pool(name="psS", bufs=2, space="PSUM") as psS,
            tc.tile_pool(name="psO", bufs=4, space="PSUM") as psO,
        ):
            # ---- tiny warm tile: ready ~immediately so the PE ramps while
            # the input DMAs are still in flight ------------------------------
            warm = cst.tile([128, 640], BF16)
            nc.vector.memset(warm[:], 0.5)
            for wi in range(N_WARM):
                pw = psS.tile([128, 1024], F32, tag="ps", name="pw")
                nc.tensor.matmul(pw[:, 0:512], warm[:, 0:128], warm[:, 128:640],
                                 start=True, stop=True)

            # ---- constant loads ---------------------------------------------
            xt = cst.tile([128, ET, S], BF16)          # x[b]^T  (e on partitions)
            wt = cst.tile([128, ET, 3 * GC], BF16)     # W_qkv slice (e on partitions)
            wpt = cst.tile([128, 2, E], BF16)          # W_proj slice (c on partitions)
            mk = cst.tile([128, 128], BF16)            # tril(128) causal mask

            # fine-grained loads, ordered by first use.  The host lays w out
            # jt-major ([q-jt0 | k-jt0 | q-jt1 | k-jt1 | v]) so each head
            # DMA is one contiguous dispatch — DMA_DIRECT2D dispatches
            # serialize ~0.7us each on the Sync queue, so fewer dispatches
            # ahead of the critical x chunk shortens the startup head.
            wr = w[:].rearrange("(t p) j -> p t j", p=128)
            xTr = xT[:].rearrange("(t p) s -> p t s", p=128)
            nc.sync.dma_start(wt[:, 0:4, 0:256], wr[:, 0:4, 0:256])
            nc.sync.dma_start(xt[:, 0:4, 0:512], xTr[:, 0:4, 0:512])
            nc.sync.dma_start(wt[:, 4:8, 0:256], wr[:, 4:8, 0:256])
            nc.sync.dma_start(xt[:, 4:8, 0:512], xTr[:, 4:8, 0:512])
            nc.sync.dma_start(wt[:, :, 512:768], wr[:, :, 512:768])
            nc.sync.dma_start(wt[:, :, 256:512], wr[:, :, 256:512])
            nc.sync.dma_start(mk[:], mask[:])
            nc.sync.dma_start(xt[:, :, 512:1024], xTr[:, :, 512:1024])
            nc.sync.dma_start(xt[:, :, 1024:1536], xTr[:, :, 1024:1536])
            nc.sync.dma_start(wpt[:], wp[:].rearrange("(t p) e -> p t e", p=128))
            nc.sync.dma_start(xt[:, :, 1536:2048], xTr[:, :, 1536:2048])

            # ---- activation buffers -----------------------------------------
            # qt/kt: [pair, j(128 part: head 2p on 0-63, head 2p+1 on 64-127), s]
            qt = acts.tile([128, 2, S], BF16)
            kt = acts.tile([128, 2, S], BF16)
            # v_aug: per s-tile, per head 65 cols (64 data + ones)
            vt = acts.tile([128, NT, HL * 65], BF16)
            # attention output^T, proj lhsT layout: c on partitions
            ot = acts.tile([128, 2, S], BF16)

            # only the ones-columns need the memset (v_chunk fills the rest)
            vones = vt[:].rearrange("p t (h m) -> p t h m", h=HL)[:, :, :, 64:65]
            nc.vector.memset(vones, 1.0)

            # ---- QKV micro-pieces ------------------------------------------
            # One qk dest (q or k of one (chunk, pair)) = an 8-matmul PSUM
            # accumulation + copy, split into two ~1us pieces sharing the
            # PSUM tile.
            def qk_micro(sc, jt, which, dest):
                s0 = 512 * sc
                st8 = {}

                def a():
                    ps = psO.tile([128, 512], F32, tag="po", name="psqk")
                    st8["ps"] = ps
                    for et in range(4):
                        nc.tensor.matmul(
                            ps[:, 0:512],
                            wt[:, et, 256 * jt + 128 * which:256 * jt + 128 * (which + 1)],
                            xt[:, et, s0:s0 + 512],
                            start=(et == 0), stop=False,
                        )

                def b():
                    ps = st8["ps"]
                    for et in range(4, ET):
                        nc.tensor.matmul(
                            ps[:, 0:512],
                            wt[:, et, 256 * jt + 128 * which:256 * jt + 128 * (which + 1)],
                            xt[:, et, s0:s0 + 512],
                            start=False, stop=(et == ET - 1),
                        )
                    nc.vector.tensor_copy(dest[:, jt, s0:s0 + 512], ps[:, 0:512])

                return [a, b]

            def qk_pieces(sc, jt):
                return qk_micro(sc, jt, 0, qt) + qk_micro(sc, jt, 1, kt)

            def v_micro(st):
                def f():
                    ps = psO.tile([128, 512], F32, tag="po", name="psv")
                    for et in range(ET):
                        nc.tensor.matmul(
                            ps[:, 0:256],
                            xt[:, et, 128 * st:128 * (st + 1)],
                            wt[:, et, 512:768],
                            start=(et == 0),
                            stop=(et == ET - 1),
                        )
                    nc.vector.tensor_copy(
                        vt[:, st].rearrange("p (h m) -> p h m", h=HL)[:, :, 0:64],
                        ps[:, 0:256].rearrange("p (h m) -> p h m", h=HL),
                    )
                return f

            def v_pieces(sc, lo=0, hi=4):
                return [v_micro(4 * sc + i) for i in range(lo, hi)]

            # ---- projection micro-piece: one 128-row s-tile -> y ----------
            def proj_micro(st, split_copy=False):
                def f():
                    ys = ysbp.tile([128, 1024], BF16)
                    for nk in range(2):
                        py = psO.tile([128, 512], F32, tag="po", name="py")
                        for ct in range(2):
                            nc.tensor.matmul(
                                py[:],
                                ot[:, ct, 128 * st:128 * (st + 1)],
                                wpt[:, ct, 512 * nk:512 * (nk + 1)],
                                start=(ct == 0),
                                stop=(ct == 1),
                            )
                        # at the very end ACT is idle: split the two copies
                        # across engines to halve the serial drain
                        if split_copy and nk == 1:
                            nc.scalar.copy(ys[:, 512 * nk:512 * (nk + 1)], py[:])
                        else:
                            nc.vector.tensor_copy(ys[:, 512 * nk:512 * (nk + 1)], py[:])
                    nc.sync.dma_start(y[128 * st:128 * (st + 1), :], ys[:])
                return f

            def proj_pieces(jq):
                return [proj_micro(4 * jq + i) for i in range(4)]

            # ---- one attention k-tile: scores -> exp -> (mask) -> attV ----
            def attn_tile(pr, jq, ik, o_ab, nik):
                s0 = 512 * jq
                t = ik - 4 * jq
                c0 = 128 * t if t > 0 else 0   # exact-causal column trim
                ps = psS.tile([128, 1024], F32)
                for ab in range(2):
                    p0 = 64 * ab
                    nc.tensor.matmul(
                        ps[:, 512 * ab + c0:512 * (ab + 1)],
                        kt[p0:p0 + 64, pr, 128 * ik:128 * (ik + 1)],
                        qt[p0:p0 + 64, pr, s0 + c0:s0 + 512],
                        start=True,
                        stop=True,
                        tile_position=(p0, 0),
                    )
                e = expp.tile([128, 1024], BF16, tag="exps", name="exps")
                e3 = e[:].rearrange("p (h n) -> p h n", h=2)[:, :, c0:512]
                ps3 = ps[:].rearrange("p (h n) -> p h n", h=2)[:, :, c0:512]
                nc.scalar.activation(e3, ps3, mybir.ActivationFunctionType.Exp,
                                     scale=float(SCALE))
                if t >= 0:
                    # only the first 128 cols of the trimmed range are
                    # partially masked; the rest is fully unmasked
                    for ab in range(2):
                        nc.vector.tensor_mul(
                            e[:, 512 * ab + c0:512 * ab + c0 + 128],
                            e[:, 512 * ab + c0:512 * ab + c0 + 128],
                            mk[:],
                        )
                return e, c0

            def attv(pr, ik, o_ab, e, ab, clo, chi, start, stop):
                h = 2 * pr + ab
                nc.tensor.matmul(
                    o_ab[ab][0:65, clo:chi],
                    vt[:, ik, 65 * h:65 * h + 65],
                    e[:, 512 * ab + clo:512 * ab + chi],
                    start=start,
                    stop=stop,
                    skip_group_check=True,
                )

            # normalize cols [clo:chi): ot[d, s] = o[d, s] / o[64, s].
            # By default the PSUM accumulator is staged out through two
            # parallel reads (ACT copies rows 0-64, DVE copies the
            # denominator row) so the PSUM slot frees ~0.8us after the last
            # attV instead of after the whole chain — the next phase's attV
            # reuses it.  final=True skips staging (nothing follows).
            # Engines can shift partitions between in and out APs, so head
            # ab=1 writes ot parts 64-127 directly.
            def attn_norm(pr, jq, o_ab, clo=0, chi=512, final=False):
                s0 = 512 * jq
                n = chi - clo
                for ab in range(2):
                    o = o_ab[ab]
                    rz = small.tile([128, 512], F32, tag="rz", name="rz")
                    nc.vector.tensor_copy(rz[0:1, 0:n], o[64:65, clo:chi])
                    if final:
                        osrc = o[0:64, clo:chi]
                    else:
                        # stage o out of PSUM; split the two copies across
                        # ACT and DVE so neither queue eats both
                        osb = small.tile([128, 512], F32, tag="osb", name="osb")
                        if ab == 0:
                            nc.scalar.copy(osb[0:64, 0:n], o[0:64, clo:chi])
                        else:
                            nc.vector.tensor_copy(osb[0:64, 0:n], o[0:64, clo:chi])
                        osrc = osb[0:64, 0:n]
                    rb = small.tile([128, 512], F32, tag="rb", name="rb")
                    nc.gpsimd.partition_broadcast(rb[0:64, 0:n], rz[0:1, 0:n])
                    rinv = small.tile([128, 512], F32, tag="rinv", name="rinv")
                    nc.vector.reciprocal_approx_fast(rinv[0:64, 0:n], rb[0:64, 0:n])
                    nc.vector.tensor_mul(
                        ot[64 * ab:64 * ab + 64, pr, s0 + clo:s0 + chi],
                        osrc, rinv[0:64, 0:n])

            # ---- attention for one (pair, q-chunk), with filler micro-
            # pieces interleaved between k-tiles.  vfill pieces (v-chunk
            # tiles with in-phase deadlines) go at the earliest slots; the
            # rest spread evenly so the phase end — where the next phase's
            # exp latency would otherwise stall the PE — stays covered. ----
            # after_first: the previous phase's deferred normalize — issued
            # right after this phase's first exp so the exp (which gates the
            # PE) gets queue priority on ACT over the staging copies.
            def attn_full(pr, jq, fillers, vfill=(), after_first=None):
                nik = 4 * jq + 4
                o_ab = [psO.tile([128, 512], F32, tag="po", name="o_ab")
                        for _ in range(2)]
                sched = {ik: [] for ik in range(nik)}
                for i, f in enumerate(vfill):
                    sched[i].append(f)
                fill = list(fillers)
                for i, f in enumerate(fill):
                    sched[min(nik - 1, int((i + 0.5) * nik / len(fill)))].append(f)
                for ik in range(nik):
                    e, c0 = attn_tile(pr, jq, ik, o_ab, nik)
                    if ik == 0 and after_first is not None:
                        after_first()
                    for ab in range(2):
                        attv(pr, ik, o_ab, e, ab, c0, 512,
                             start=(ik == 0), stop=(ik == nik - 1))
                    for f in sched[ik]:
                        f()
                return lambda: attn_norm(pr, jq, o_ab)

            # ---- final chunk (pr=1, jq=3): split attV into q-halves (with
            # separate PSUM tiles, so accumulation groups and the normalize
            # reads have clean tile-level dependencies) so the normalize
            # chains overlap remaining matmul work.  The low-half normalize
            # frees its PSUM slots early, making room for the st 12/13
            # projection pieces to fill the exp-bound last k-tiles.
            def attn_last(pr, jq, after_first=None):
                nik = 4 * jq + 4        # 16
                mid = nik - 1           # k-tile where the low 3/4 closes
                o_lo = [psO.tile([128, 512], F32, tag="po", name="o_ab")
                        for _ in range(2)]
                o_hi = [psO.tile([128, 512], F32, tag="po", name="o_ab")
                        for _ in range(2)]
                for ik in range(nik):
                    e, c0 = attn_tile(pr, jq, ik, None, nik)
                    if ik == 0 and after_first is not None:
                        after_first()
                    for ab in range(2):
                        if ik < mid and c0 < 384:
                            attv(pr, ik, o_lo, e, ab, c0, 384,
                                 start=(ik == 0), stop=(ik == mid - 1))
                        if ik < mid:
                            attv(pr, ik, o_hi, e, ab, max(384, c0), 512,
                                 start=(ik == 0), stop=False)
                        else:
                            attv(pr, ik, o_hi, e, ab, c0, 512,
                                 start=False, stop=(ik == nik - 1))
                    if ik == mid - 1:
                        # low-3/4 normalize: hides under the last k-tile
                        attn_norm(pr, jq, o_lo, 0, 384)
                # st 12-14 projection overlaps the final normalize chain
                proj_micro(4 * jq + 0)()
                attn_norm(pr, jq, o_hi, 384, 512, final=True)
                proj_micro(4 * jq + 1)()
                proj_micro(4 * jq + 2, split_copy=True)()

            # ---- schedule -------------------------------------------------
            qk_pieces_00 = qk_pieces(0, 0)
            for p in qk_pieces_00:
                p()
            v_micro(0)()
            nrm = attn_full(0, 0, qk_pieces(0, 1), vfill=v_pieces(0, 1, 4))
            nrm = attn_full(1, 0, qk_pieces(1, 0) + qk_pieces(1, 1),
                            after_first=nrm)
            nrm = attn_full(0, 1, proj_pieces(0), vfill=v_pieces(1),
                            after_first=nrm)
            nrm = attn_full(1, 1, qk_pieces(2, 0), after_first=nrm)
            nrm = attn_full(0, 2, qk_pieces(2, 1), vfill=v_pieces(2),
                            after_first=nrm)
            nrm = attn_full(1, 2, qk_pieces(3, 0) + proj_pieces(1),
                            after_first=nrm)
            nrm = attn_full(0, 3, qk_pieces(3, 1) + proj_pieces(2),
                            vfill=v_pieces(3), after_first=nrm)
            attn_last(1, 3, after_first=nrm)
            proj_micro(15, split_copy=True)()

    nc.compile()
    return nc


def _get_nc():
    global _CACHED_NC
    if _CACHED_NC is None:
        _CACHED_NC = _build()
    return _CACHED_NC


def _diag_masks() -> np.ndarray:
    return np.ascontiguousarray(
        np.tril(np.ones((128, 128), dtype=np.float32)).T).astype(NP_BF16)


def _in_maps(x, W_qkv, W_proj):
    masks = _diag_masks()
    maps = []
    for c in range(N_CORES):
        b, g = divmod(c, 4)
        xT = np.ascontiguousarray(x[b].T).astype(NP_BF16)
        wq = W_qkv[:, GC * g:GC * (g + 1)]
        wk = W_qkv[:, E + GC * g:E + GC * (g + 1)]
        wv = W_qkv[:, 2 * E + GC * g:2 * E + GC * (g + 1)]
        # jt-major: [q-jt0 | k-jt0 | q-jt1 | k-jt1 | v] so the first-needed
        # weight columns load as single contiguous DMAs
        w = np.ascontiguousarray(np.concatenate(
            [wq[:, 0:128], wk[:, 0:128], wq[:, 128:256], wk[:, 128:256], wv],
            axis=1)).astype(NP_BF16)
        wp = np.ascontiguousarray(W_proj[GC * g:GC * (g + 1), :]).astype(NP_BF16)
        maps.append({"xT": xT, "w": w, "wp": wp, "mask": masks})
    return maps


def _run(x, W_qkv, W_proj, trace=False, **spmd_kwargs):
    nc = _get_nc()
    res = run_bass_kernel_spmd(nc, _in_maps(x, W_qkv, W_proj),
                               list(range(N_CORES)), trace=trace, **spmd_kwargs)
    out = np.zeros((B, S, E), dtype=np.float32)
    for c in range(N_CORES):
        out[c // 4] += np.asarray(res.results[c]["y"]).astype(np.float32)
    return out, res


def kernel(x, attention_mask, W_qkv, W_proj):
    x = np.asarray(x, dtype=np.float32)
    W_qkv = np.asarray(W_qkv, dtype=np.float32)
    W_proj = np.asarray(W_proj, dtype=np.float32)
    out, _ = _run(x, W_qkv, W_proj, trace=False)
    return out


# revision 34
# speedup vs baseline: 1.0117x; 1.0117x over previous
"""Multi-head attention (B=2, S=2048, E=1024, H=16, causal) on 8 TRN2 cores.

Sharding: core c -> batch b = c//4, head group g = c%4 (4 heads each).
Each core computes QKV projection for its heads, causal flash-style
attention (no-max softmax, denominator via ones-column appended to V),
and a partial output projection against a 256-row slice of W_proj.
Host sums the 4 partial projections per batch (the "all-reduce") and
stacks the 2 batches.

All matmul operands are bfloat16 (fp32 accumulation in PSUM): same PE
row rate as fp32r but ~2x faster LDWEIGHTS, half the DMA bytes, half
SBUF pressure, and 2x DVE throughput on copies.

Scheduling: attention is ACT(exp)-latency-bound per score tile, and the
PE queue is in-order — so independent QKV/projection matmul work is
chopped into ~1-2us micro-pieces and interleaved after (almost) every
attention k-tile, keeping the PE dense and HAM-warm.  The final
attention chunk splits its attV accumulation into two q-halves so the
softmax-normalize chain (PSUM row copy -> gpsimd partition broadcast ->
reciprocal -> scale) of each half hides under remaining matmul work.
"""
import sys

sys.path.insert(0, "/opt/trn_rl_repo")

import numpy as np
import ml_dtypes

import concourse.bacc as bacc
import concourse.mybir as mybir
from concourse import tile
from concourse.bass_utils import run_bass_kernel_spmd

B, S, E, H, D = 2, 2048, 1024, 16, 64
SCALE = D ** -0.5
N_CORES = 8
HL = 4            # heads per core
GC = 256          # channel columns per core (HL * D)
F32 = mybir.dt.float32
BF16 = mybir.dt.bfloat16
NP_BF16 = ml_dtypes.bfloat16

_CACHED_NC = None

N_WARM = 14       # warmup matmuls to ramp PE p-state during the DMA head


def _build():
    nc = bacc.Bacc("TRN2", target_bir_lowering=False, debug=False,
                   num_devices=N_CORES)

    xT = nc.dram_tensor("xT", [E, S], BF16, kind="ExternalInput")
    w = nc.dram_tensor("w", [E, 3 * GC], BF16, kind="ExternalInput")
    wp = nc.dram_tensor("wp", [GC, E], BF16, kind="ExternalInput")
    mask = nc.dram_tensor("mask", [128, 128], BF16, kind="ExternalInput")
    y = nc.dram_tensor("y", [S, E], BF16, kind="ExternalOutput")

    ET = E // 128     # 8 e-tiles
    NS = S // 512     # 4 s-chunks of 512
    NT = S // 128     # 16 s-tiles of 128

    with tile.TileContext(nc) as tc:
        with (
            tc.tile_pool(name="const", bufs=1) as cst,
            tc.tile_pool(name="acts", bufs=1) as acts,
            tc.tile_pool(name="expp", bufs=8) as expp,
            tc.tile_pool(name="small", bufs=2) as small,
            tc.tile_pool(name="ysb", bufs=3) as ysbp,
            tc.tile_pool(name="psS", bufs=2, space="PSUM") as psS,
            tc.tile_pool(name="psO", bufs=4, space="PSUM") as psO,
        ):
            # ---- tiny warm tile: ready ~immediately so the PE ramps while
            # the input DMAs are still in flight ------------------------------
            warm = cst.tile([128, 640], BF16)
            nc.vector.memset(warm[:], 0.5)
            for wi in range(N_WARM):
                pw = psS.tile([128, 1024], F32, tag="ps", name="pw")
                nc.tensor.matmul(pw[:, 0:512], warm[:, 0:128], warm[:, 128:640],
                                 start=True, stop=True)

            # ---- constant loads ---------------------------------------------
            xt = cst.tile([128, ET, S], BF16)          # x[b]^T  (e on partitions)
            wt = cst.tile([128, ET, 3 * GC], BF16)     # W_qkv slice (e on partitions)
            wpt = cst.tile([128, 2, E], BF16)          # W_proj slice (c on partitions)
            mk = cst.tile([128, 128], BF16)            # tril(128) causal mask

            # fine-grained loads, ordered by first use.  The host lays w out
            # jt-major ([q-jt0 | k-jt0 | q-jt1 | k-jt1 | v]) so each head
            # DMA is one contiguous dispatch — DMA_DIRECT2D dispatches
            # serialize ~0.7us each on the Sync queue, so fewer dispatches
            # ahead of the critical x chunk shortens the startup head.
            wr = w[:].rearrange("(t p) j -> p t j", p=128)
            xTr = xT[:].rearrange("(t p) s -> p t s", p=128)
            nc.sync.dma_start(wt[:, 0:4, 0:256], wr[:, 0:4, 0:256])
            nc.sync.dma_start(xt[:, 0:4, 0:512], xTr[:, 0:4, 0:512])
            nc.sync.dma_start(wt[:, 4:8, 0:256], wr[:, 4:8, 0:256])
            nc.sync.dma_start(xt[:, 4:8, 0:512], xTr[:, 4:8, 0:512])
            nc.sync.dma_start(wt[:, :, 512:768], wr[:, :, 512:768])
            nc.sync.dma_start(wt[:, :, 256:512], wr[:, :, 256:512])
            nc.sync.dma_start(mk[:], mask[:])
            nc.sync.dma_start(xt[:, :, 512:1024], xTr[:, :, 512:1024])
            nc.sync.dma_start(xt[:, :, 1024:1536], xTr[:, :, 1024:1536])
            nc.sync.dma_start(wpt[:], wp[:].rearrange("(t p) e -> p t e", p=128))
            nc.sync.dma_start(xt[:, :, 1536:2048], xTr[:, :, 1536:2048])

            # ---- activation buffers -----------------------------------------
            # qt/kt: [pair, j(128 part: head 2p on 0-63, head 2p+1 on 64-127), s]
            qt = acts.tile([128, 2, S], BF16)
            kt = acts.tile([128, 2, S], BF16)
            # v_aug: per s-tile, per head 65 cols (64 data + ones)
            vt = acts.tile([128, NT, HL * 65], BF16)
            # attention output^T, proj lhsT layout: c on partitions
            ot = acts.tile([128, 2, S], BF16)

            # only the ones-columns need the memset (v_chunk fills the rest)
            vones = vt[:].rearrange("p t (h m) -> p t h m", h=HL)[:, :, :, 64:65]
            nc.vector.memset(vones, 1.0)

            # ---- QKV micro-pieces ------------------------------------------
            # One qk dest (q or k of one (chunk, pair)) = an 8-matmul PSUM
            # accumulation + copy, split into two ~1us pieces sharing the
            # PSUM tile.
            def qk_micro(sc, jt, which, dest):
                s0 = 512 * sc
                st8 = {}

                def a():
                    ps = psO.tile([128, 512], F32, tag="po", name="psqk")
                    st8["ps"] = ps
                    for et in range(4):
                        nc.tensor.matmul(
                            ps[:, 0:512],
                            wt[:, et, 256 * jt + 128 * which:256 * jt + 128 * (which + 1)],
                            xt[:, et, s0:s0 + 512],
                            start=(et == 0), stop=False,
                        )

                def b():
                    ps = st8["ps"]
                    for et in range(4, ET):
                        nc.tensor.matmul(
                            ps[:, 0:512],
                            wt[:, et, 256 * jt + 128 * which:256 * jt + 128 * (which + 1)],
                            xt[:, et, s0:s0 + 512],
                            start=False, stop=(et == ET - 1),
                        )
                    nc.vector.tensor_copy(dest[:, jt, s0:s0 + 512], ps[:, 0:512])

                return [a, b]

            def qk_pieces(sc, jt):
                return qk_micro(sc, jt, 0, qt) + qk_micro(sc, jt, 1, kt)

            def v_micro(st):
                def f():
                    ps = psO.tile([128, 512], F32, tag="po", name="psv")
                    for et in range(ET):
                        nc.tensor.matmul(
                            ps[:, 0:256],
                            xt[:, et, 128 * st:128 * (st + 1)],
                            wt[:, et, 512:768],
                            start=(et == 0),
                            stop=(et == ET - 1),
                        )
                    nc.vector.tensor_copy(
                        vt[:, st].rearrange("p (h m) -> p h m", h=HL)[:, :, 0:64],
                        ps[:, 0:256].rearrange("p (h m) -> p h m", h=HL),
                    )
                return f

            def v_pieces(sc, lo=0, hi=4):
                return [v_micro(4 * sc + i) for i in range(lo, hi)]

            # ---- projection micro-piece: one 128-row s-tile -> y ----------
            def proj_micro(st, split_copy=False, half_dma=False):
                def f():
                    ys = ysbp.tile([128, 1024], BF16)
                    for nk in range(2):
                        py = psO.tile([128, 512], F32, tag="po", name="py")
                        for ct in range(2):
                            nc.tensor.matmul(
                                py[:],
                                ot[:, ct, 128 * st:128 * (st + 1)],
                                wpt[:, ct, 512 * nk:512 * (nk + 1)],
                                start=(ct == 0),
                                stop=(ct == 1),
                            )
                        # at the very end ACT is idle: split the two copies
                        # across engines to halve the serial drain
                        if split_copy and nk == 1:
                            nc.scalar.copy(ys[:, 512 * nk:512 * (nk + 1)], py[:])
                        else:
                            nc.vector.tensor_copy(ys[:, 512 * nk:512 * (nk + 1)], py[:])
                        # for the final tiles, ship each half as soon as its
                        # copy lands so the transfer overlaps the other copy
                        if half_dma:
                            nc.sync.dma_start(
                                y[128 * st:128 * (st + 1), 512 * nk:512 * (nk + 1)],
                                ys[:, 512 * nk:512 * (nk + 1)])
                    if not half_dma:
                        nc.sync.dma_start(y[128 * st:128 * (st + 1), :], ys[:])
                return f

            def proj_pieces(jq):
                return [proj_micro(4 * jq + i) for i in range(4)]

            # ---- one attention k-tile: scores -> exp -> (mask) -> attV ----
            def attn_tile(pr, jq, ik, o_ab, nik):
                s0 = 512 * jq
                t = ik - 4 * jq
                c0 = 128 * t if t > 0 else 0   # exact-causal column trim
                ps = psS.tile([128, 1024], F32)
                for ab in range(2):
                    p0 = 64 * ab
                    nc.tensor.matmul(
                        ps[:, 512 * ab + c0:512 * (ab + 1)],
                        kt[p0:p0 + 64, pr, 128 * ik:128 * (ik + 1)],
                        qt[p0:p0 + 64, pr, s0 + c0:s0 + 512],
                        start=True,
                        stop=True,
                        tile_position=(p0, 0),
                    )
                e = expp.tile([128, 1024], BF16, tag="exps", name="exps")
                e3 = e[:].rearrange("p (h n) -> p h n", h=2)[:, :, c0:512]
                ps3 = ps[:].rearrange("p (h n) -> p h n", h=2)[:, :, c0:512]
                nc.scalar.activation(e3, ps3, mybir.ActivationFunctionType.Exp,
                                     scale=float(SCALE))
                if t >= 0:
                    # only the first 128 cols of the trimmed range are
                    # partially masked; the rest is fully unmasked
                    for ab in range(2):
                        nc.vector.tensor_mul(
                            e[:, 512 * ab + c0:512 * ab + c0 + 128],
                            e[:, 512 * ab + c0:512 * ab + c0 + 128],
                            mk[:],
                        )
                return e, c0

            def attv(pr, ik, o_ab, e, ab, clo, chi, start, stop):
                h = 2 * pr + ab
                nc.tensor.matmul(
                    o_ab[ab][0:65, clo:chi],
                    vt[:, ik, 65 * h:65 * h + 65],
                    e[:, 512 * ab + clo:512 * ab + chi],
                    start=start,
                    stop=stop,
                    skip_group_check=True,
                )

            # normalize cols [clo:chi): ot[d, s] = o[d, s] / o[64, s].
            # By default the PSUM accumulator is staged out through two
            # parallel reads (ACT copies rows 0-64, DVE copies the
            # denominator row) so the PSUM slot frees ~0.8us after the last
            # attV instead of after the whole chain — the next phase's attV
            # reuses it.  final=True skips staging (nothing follows).
            # Engines can shift partitions between in and out APs, so head
            # ab=1 writes ot parts 64-127 directly.
            def attn_norm(pr, jq, o_ab, clo=0, chi=512, final=False):
                s0 = 512 * jq
                n = chi - clo
                for ab in range(2):
                    o = o_ab[ab]
                    rz = small.tile([128, 512], F32, tag="rz", name="rz")
                    nc.vector.tensor_copy(rz[0:1, 0:n], o[64:65, clo:chi])
                    if final:
                        osrc = o[0:64, clo:chi]
                    else:
                        # stage o out of PSUM; split the two copies across
                        # ACT and DVE so neither queue eats both
                        osb = small.tile([128, 512], F32, tag="osb", name="osb")
                        if ab == 0:
                            nc.scalar.copy(osb[0:64, 0:n], o[0:64, clo:chi])
                        else:
                            nc.vector.tensor_copy(osb[0:64, 0:n], o[0:64, clo:chi])
                        osrc = osb[0:64, 0:n]
                    rb = small.tile([128, 512], F32, tag="rb", name="rb")
                    nc.gpsimd.partition_broadcast(rb[0:64, 0:n], rz[0:1, 0:n])
                    rinv = small.tile([128, 512], F32, tag="rinv", name="rinv")
                    nc.vector.reciprocal_approx_fast(rinv[0:64, 0:n], rb[0:64, 0:n])
                    nc.vector.tensor_mul(
                        ot[64 * ab:64 * ab + 64, pr, s0 + clo:s0 + chi],
                        osrc, rinv[0:64, 0:n])

            # ---- attention for one (pair, q-chunk), with filler micro-
            # pieces interleaved between k-tiles.  vfill pieces (v-chunk
            # tiles with in-phase deadlines) go at the earliest slots; the
            # rest spread evenly so the phase end — where the next phase's
            # exp latency would otherwise stall the PE — stays covered. ----
            # after_first: the previous phase's deferred normalize — issued
            # right after this phase's first exp so the exp (which gates the
            # PE) gets queue priority on ACT over the staging copies.
            def attn_full(pr, jq, fillers, vfill=(), after_first=None):
                nik = 4 * jq + 4
                o_ab = [psO.tile([128, 512], F32, tag="po", name="o_ab")
                        for _ in range(2)]
                sched = {ik: [] for ik in range(nik)}
                for i, f in enumerate(vfill):
                    sched[i].append(f)
                fill = list(fillers)
                for i, f in enumerate(fill):
                    sched[min(nik - 1, int((i + 0.5) * nik / len(fill)))].append(f)
                for ik in range(nik):
                    e, c0 = attn_tile(pr, jq, ik, o_ab, nik)
                    if ik == 0 and after_first is not None:
                        after_first()
                    for ab in range(2):
                        attv(pr, ik, o_ab, e, ab, c0, 512,
                             start=(ik == 0), stop=(ik == nik - 1))
                    for f in sched[ik]:
                        f()
                return lambda: attn_norm(pr, jq, o_ab)

            # ---- final chunk (pr=1, jq=3): split attV into q-halves (with
            # separate PSUM tiles, so accumulation groups and the normalize
            # reads have clean tile-level dependencies) so the normalize
            # chains overlap remaining matmul work.  The low-half normalize
            # frees its PSUM slots early, making room for the st 12/13
            # projection pieces to fill the exp-bound last k-tiles.
            def attn_last(pr, jq, after_first=None):
                nik = 4 * jq + 4        # 16
                mid = nik - 1           # k-tile where the low 3/4 closes
                o_lo = [psO.tile([128, 512], F32, tag="po", name="o_ab")
                        for _ in range(2)]
                o_hi = [psO.tile([128, 512], F32, tag="po", name="o_ab")
                        for _ in range(2)]
                for ik in range(nik):
                    e, c0 = attn_tile(pr, jq, ik, None, nik)
                    if ik == 0 and after_first is not None:
                        after_first()
                    for ab in range(2):
                        if ik < mid and c0 < 384:
                            attv(pr, ik, o_lo, e, ab, c0, 384,
                                 start=(ik == 0), stop=(ik == mid - 1))
                        if ik < mid:
                            attv(pr, ik, o_hi, e, ab, max(384, c0), 512,
                                 start=(ik == 0), stop=False)
                        else:
                            attv(pr, ik, o_hi, e, ab, c0, 512,
                                 start=False, stop=(ik == nik - 1))
                    if ik == mid - 1:
                        # low-3/4 normalize: hides under the last k-tile
                        attn_norm(pr, jq, o_lo, 0, 384)
                # st 12-14 projection overlaps the final normalize chain
                proj_micro(4 * jq + 0)()
                attn_norm(pr, jq, o_hi, 384, 512, final=True)
                proj_micro(4 * jq + 1, split_copy=True)()
                proj_micro(4 * jq + 2, split_copy=True, half_dma=True)()

            # ---- schedule -------------------------------------------------
            qk_pieces_00 = qk_pieces(0, 0)
            for p in qk_pieces_00:
                p()
            v_micro(0)()
            nrm = attn_full(0, 0, qk_pieces(0, 1), vfill=v_pieces(0, 1, 4))
            nrm = attn_full(1, 0, qk_pieces(1, 0) + qk_pieces(1, 1),
                            after_first=nrm)
            nrm = attn_full(0, 1, proj_pieces(0), vfill=v_pieces(1),
                            after_first=nrm)
            nrm = attn_full(1, 1, qk_pieces(2, 0), after_first=nrm)
            nrm = attn_full(0, 2, qk_pieces(2, 1), vfill=v_pieces(2),
                            after_first=nrm)
            nrm = attn_full(1, 2, qk_pieces(3, 0) + proj_pieces(1),
                            after_first=nrm)
            nrm = attn_full(0, 3, qk_pieces(3, 1) + proj_pieces(2),
                            vfill=v_pieces(3), after_first=nrm)
            attn_last(1, 3, after_first=nrm)
            proj_micro(15, split_copy=True, half_dma=True)()

    nc.compile()
    return nc


def _get_nc():
    global _CACHED_NC
    if _CACHED_NC is None:
        _CACHED_NC = _build()
    return _CACHED_NC


def _diag_masks() -> np.ndarray:
    return np.ascontiguousarray(
        np.tril(np.ones((128, 128), dtype=np.float32)).T).astype(NP_BF16)


def _in_maps(x, W_qkv, W_proj):
    masks = _diag_masks()
    maps = []
    for c in range(N_CORES):
        b, g = divmod(c, 4)
        xT = np.ascontiguousarray(x[b].T).astype(NP_BF16)
        wq = W_qkv[:, GC * g:GC * (g + 1)]
        wk = W_qkv[:, E + GC * g:E + GC * (g + 1)]
        wv = W_qkv[:, 2 * E + GC * g:2 * E + GC * (g + 1)]
        # jt-major: [q-jt0 | k-jt0 | q-jt1 | k-jt1 | v] so the first-needed
        # weight columns load as single contiguous DMAs
        w = np.ascontiguousarray(np.concatenate(
            [wq[:, 0:128], wk[:, 0:128], wq[:, 128:256], wk[:, 128:256], wv],
            axis=1)).astype(NP_BF16)
        wp = np.ascontiguousarray(W_proj[GC * g:GC * (g + 1), :]).astype(NP_BF16)
        maps.append({"xT": xT, "w": w, "wp": wp, "mask": masks})
    return maps


def _run(x, W_qkv, W_proj, trace=False, **spmd_kwargs):
    nc = _get_nc()
    res = run_bass_kernel_spmd(nc, _in_maps(x, W_qkv, W_proj),
                               list(range(N_CORES)), trace=trace, **spmd_kwargs)
    out = np.zeros((B, S, E), dtype=np.float32)
    for c in range(N_CORES):
        out[c // 4] += np.asarray(res.results[c]["y"]).astype(np.float32)
    return out, res


def kernel(x, attention_mask, W_qkv, W_proj):
    x = np.asarray(x, dtype=np.float32)
    W_qkv = np.asarray(W_qkv, dtype=np.float32)
    W_proj = np.asarray(W_proj, dtype=np.float32)
    out, _ = _run(x, W_qkv, W_proj, trace=False)
    return out
